# revision 4
# baseline (speedup 1.0000x reference)
"""Trainium2 Bass kernel for a small MoE layer (4 routed experts top-2 + 2 shared).

Strategy: data-parallel over tokens across 8 NeuronCores. Each core gets
1024 tokens and computes the full MoE for them:
  - gating (softmax + top-2 mask) in true fp32 so expert selection matches
    the fp32 reference,
  - all 6 expert MLPs (2 shared, 4 routed) computed densely with bf16
    matmul inputs and fp32 PSUM accumulation,
  - routed expert outputs weighted per-token by the masked softmax probs,
    shared experts averaged; accumulated in fp32.

Layouts (per core):
  x^T resident in SBUF as 8 chunks [128(D), 1024(tok)]
  L1: psum[128(F),512(tok)] = sum_d w1[d,:,fchunk].T @ x[d]   (lhsT = w1 chunk)
  h  : [128(F), 1024(tok)] bf16 via ACT relu(psum + b1)
  L2: psum[128(tok),512(O)] = sum_f h[f][:,tokchunk].T @ w2[f]  (lhsT = h chunk)
  out: [128(tok), 1024(O)] fp32, accumulated via ACT scale-copy + DVE add.
"""

import sys

sys.path.insert(0, '/opt/trn_rl_repo')

import numpy as np
import ml_dtypes

import concourse.bass as bass
import concourse.mybir as mybir
import concourse.tile as tile
from concourse import bacc
from concourse.bass_utils import run_bass_kernel_spmd

BF16 = ml_dtypes.bfloat16

NCORES = 8
B, S, D, F, O = 4, 2048, 1024, 4096, 1024
E, NS, KTOP = 4, 2, 2
NEXP = NS + E            # 6 MLPs: shared first, then routed
T = (B * S) // NCORES    # 1024 tokens per core
P = 128
DCH = D // P             # 8
FCH = F // P             # 32
TCH = T // P             # 8
FBLK_CH = 8              # F-chunks per block
NFBLK = FCH // FBLK_CH   # 4
NTH = T // 512           # 2 token halves (512-wide matmul moving dim)
NOH = O // 512           # 2 output halves

_CACHED = None


def _build():
    f32 = mybir.dt.float32
    bf = mybir.dt.bfloat16
    AF = mybir.ActivationFunctionType
    ALU = mybir.AluOpType
    AX = mybir.AxisListType

    nc = bacc.Bacc("TRN2", target_bir_lowering=False, debug=False)

    x32_d = nc.dram_tensor("x32", [DCH, P, T], f32, kind="ExternalInput")
    xb_d = nc.dram_tensor("xb", [DCH, P, T], bf, kind="ExternalInput")
    w1_d = nc.dram_tensor("w1", [NEXP, DCH, P, F], bf, kind="ExternalInput")
    w2_d = nc.dram_tensor("w2", [NEXP, FCH, P, O], bf, kind="ExternalInput")
    b1_d = nc.dram_tensor("b1", [NEXP, P, FCH], f32, kind="ExternalInput")
    b2_d = nc.dram_tensor("b2", [NEXP, 1, O], bf, kind="ExternalInput")
    gw_d = nc.dram_tensor("gw", [DCH, P, E], f32, kind="ExternalInput")
    gb_d = nc.dram_tensor("gb", [1, E], f32, kind="ExternalInput")
    out_d = nc.dram_tensor("out", [T, O], f32, kind="ExternalOutput")

    with tile.TileContext(nc) as tc:
        with (
            tc.tile_pool(name="xres", bufs=1) as xres,
            tc.tile_pool(name="xbres", bufs=1) as xbres,
            tc.tile_pool(name="outres", bufs=1) as outres,
            tc.tile_pool(name="consts", bufs=1) as consts,
            tc.tile_pool(name="gsb", bufs=2) as gsb,
            tc.tile_pool(name="cres", bufs=1) as cres,
            tc.tile_pool(name="w1p", bufs=12) as w1p,
            tc.tile_pool(name="w2p", bufs=10) as w2p,
            tc.tile_pool(name="hp", bufs=10) as hp,
            tc.tile_pool(name="tmp", bufs=4) as tmpp,
            tc.tile_pool(name="gps", bufs=2, space="PSUM") as gps,
            tc.tile_pool(name="hps", bufs=3, space="PSUM") as hps,
            tc.tile_pool(name="yps", bufs=3, space="PSUM") as yps,
        ):
            # ---- resident loads ----
            x32 = []
            xb = []
            for d in range(DCH):
                t32 = xres.tile([P, T], f32, tag=f"x32_{d}", name=f"x32_{d}")
                nc.sync.dma_start(t32[:], x32_d[d])
                x32.append(t32)
                tb = xbres.tile([P, T], bf, tag=f"xb_{d}", name=f"xb_{d}")
                nc.sync.dma_start(tb[:], xb_d[d])
                xb.append(tb)

            gw = []
            for d in range(DCH):
                g = consts.tile([P, E], f32, tag=f"gw{d}", name=f"gw{d}")
                nc.sync.dma_start(g[:], gw_d[d])
                gw.append(g)
            gb = consts.tile([1, E], f32, tag="gb", name="gb")
            nc.sync.dma_start(gb[:], gb_d[0:1, :])
            b1 = []
            b2 = []
            for e in range(NEXP):
                t1 = consts.tile([P, FCH], f32, tag=f"b1_{e}", name=f"b1_{e}")
                nc.sync.dma_start(t1[:], b1_d[e])
                b1.append(t1)
                t2 = consts.tile([1, O], bf, tag=f"b2_{e}", name=f"b2_{e}")
                nc.sync.dma_start(t2[:], b2_d[e])
                b2.append(t2)
            ones32 = consts.tile([1, P], f32, tag="ones32", name="ones32")
            nc.vector.memset(ones32[:], 1.0)
            onesbf = consts.tile([1, P], bf, tag="onesbf", name="onesbf")
            nc.vector.memset(onesbf[:], 1.0)

            # ---- gating: fp32 logits -> top2 mask on logits -> softmax -> c ----
            c_tiles = []
            for t in range(TCH):
                ps = gps.tile([P, E], f32, tag="gps", name=f"gps_{t}")
                for d in range(DCH):
                    nc.tensor.matmul(
                        ps[:], lhsT=x32[d][:, t * P:(t + 1) * P], rhs=gw[d][:],
                        start=(d == 0), stop=False)
                nc.tensor.matmul(ps[:], lhsT=ones32[:], rhs=gb[:],
                                 start=False, stop=True)
                lg = gsb.tile([P, E], f32, tag="lg", name=f"lg_{t}")
                nc.scalar.copy(lg[:], ps[:])
                m1 = gsb.tile([P, 1], f32, tag="m1", name=f"m1_{t}")
                nc.vector.tensor_reduce(m1[:], lg[:], AX.X, ALU.max)
                eq = gsb.tile([P, E], f32, tag="eq", name=f"eq_{t}")
                nc.vector.tensor_scalar(eq[:], lg[:], m1[:], None, ALU.is_ge)
                eqs = gsb.tile([P, E], f32, tag="eqs", name=f"eqs_{t}")
                nc.vector.tensor_scalar_mul(eqs[:], eq[:], -1e30)
                pm = gsb.tile([P, E], f32, tag="pm", name=f"pm_{t}")
                nc.vector.tensor_tensor(pm[:], lg[:], eqs[:], ALU.add)
                m2 = gsb.tile([P, 1], f32, tag="m2", name=f"m2_{t}")
                nc.vector.tensor_reduce(m2[:], pm[:], AX.X, ALU.max)
                keep = gsb.tile([P, E], f32, tag="keep", name=f"keep_{t}")
                nc.vector.tensor_scalar(keep[:], lg[:], m2[:], None, ALU.is_ge)
                negm = gsb.tile([P, 1], f32, tag="negm", name=f"negm_{t}")
                nc.vector.tensor_scalar_mul(negm[:], m1[:], -1.0)
                ex = gsb.tile([P, E], f32, tag="ex", name=f"ex_{t}")
                nc.scalar.activation(ex[:], lg[:], AF.Exp, bias=negm[:], scale=1.0)
                ssum = gsb.tile([P, 1], f32, tag="ssum", name=f"ssum_{t}")
                nc.vector.tensor_reduce(ssum[:], ex[:], AX.X, ALU.add)
                rcp = gsb.tile([P, 1], f32, tag="rcp", name=f"rcp_{t}")
                nc.vector.reciprocal(rcp[:], ssum[:])
                pr = gsb.tile([P, E], f32, tag="pr", name=f"pr_{t}")
                nc.vector.tensor_scalar(pr[:], ex[:], rcp[:], None, ALU.mult)
                ct = cres.tile([P, E], f32, tag=f"c_{t}", name=f"c_{t}")
                nc.vector.tensor_tensor(ct[:], pr[:], keep[:], ALU.mult)
                c_tiles.append(ct)

            # ---- expert MLPs ----
            out_sb = [outres.tile([P, O], f32, tag=f"out_{t}", name=f"out_{t}") for t in range(TCH)]

            for e in range(NEXP):
                for fb in range(NFBLK):
                    # L1 weights for this F block: [128(D), 1024(F)] per d-chunk
                    w1t = []
                    for d in range(DCH):
                        wt = w1p.tile([P, FBLK_CH * P], bf, tag="w1", name=f"w1_{e}_{fb}_{d}")
                        nc.sync.dma_start(
                            wt[:], w1_d[e, d, :, fb * FBLK_CH * P:(fb + 1) * FBLK_CH * P])
                        w1t.append(wt)
                    # L1 matmuls + relu into h (bf16)
                    h = []
                    for fc in range(FBLK_CH):
                        ht = hp.tile([P, T], bf, tag="h", name=f"h_{e}_{fb}_{fc}")
                        for th in range(NTH):
                            ph = hps.tile([P, 512], f32, tag="hps", name=f"hps_{e}_{fb}_{fc}_{th}")
                            for d in range(DCH):
                                nc.tensor.matmul(
                                    ph[:],
                                    lhsT=w1t[d][:, fc * P:(fc + 1) * P],
                                    rhs=xb[d][:, th * 512:(th + 1) * 512],
                                    start=(d == 0), stop=(d == DCH - 1))
                            fidx = fb * FBLK_CH + fc
                            nc.scalar.activation(
                                ht[:, th * 512:(th + 1) * 512], ph[:],
                                AF.Relu, bias=b1[e][:, fidx:fidx + 1], scale=1.0)
                        h.append(ht)
                    # L2 weights for this F block: [128(F), 1024(O)] per f-chunk
                    w2t = []
                    for fc in range(FBLK_CH):
                        wt = w2p.tile([P, O], bf, tag="w2", name=f"w2_{e}_{fb}_{fc}")
                        nc.sync.dma_start(wt[:], w2_d[e, fb * FBLK_CH + fc])
                        w2t.append(wt)
                    # L2 matmuls, drain with scale into out accumulators
                    last_blk = (fb == NFBLK - 1)
                    for t in range(TCH):
                        for oh in range(NOH):
                            yp = yps.tile([P, 512], f32, tag="yps", name=f"yps_{e}_{fb}_{t}_{oh}")
                            for fc in range(FBLK_CH):
                                nc.tensor.matmul(
                                    yp[:],
                                    lhsT=h[fc][:, t * P:(t + 1) * P],
                                    rhs=w2t[fc][:, oh * 512:(oh + 1) * 512],
                                    start=(fc == 0),
                                    stop=(fc == FBLK_CH - 1 and not last_blk))
                            if last_blk:
                                nc.tensor.matmul(
                                    yp[:], lhsT=onesbf[:],
                                    rhs=b2[e][:, oh * 512:(oh + 1) * 512],
                                    start=False, stop=True)
                            if e < NS:
                                scale = 0.5
                            else:
                                scale = c_tiles[t][:, (e - NS):(e - NS) + 1]
                            osl = out_sb[t][:, oh * 512:(oh + 1) * 512]
                            if e == 0 and fb == 0:
                                nc.scalar.activation(osl, yp[:], AF.Copy,
                                                     bias=0.0, scale=scale)
                            else:
                                tm = tmpp.tile([P, 512], f32, tag="tm", name=f"tm_{e}_{fb}_{t}_{oh}")
                                nc.scalar.activation(tm[:], yp[:], AF.Copy,
                                                     bias=0.0, scale=scale)
                                nc.vector.tensor_tensor(osl, osl, tm[:], ALU.add)

            for t in range(TCH):
                nc.sync.dma_start(out_d[t * P:(t + 1) * P, :], out_sb[t][:])

    nc.finalize()
    return nc


def _get_nc():
    global _CACHED
    if _CACHED is None:
        _CACHED = _build()
    return _CACHED


def _prep_inputs(x, gate_w, gate_b, sw1, sb1, sw2, sb2, rw1, rb1, rw2, rb2):
    """Host-side sharding + layout prep. Returns per-core in_maps."""
    xf = np.ascontiguousarray(np.asarray(x, np.float32).reshape(B * S, D))
    w1_all = np.concatenate([np.asarray(sw1, np.float32),
                             np.asarray(rw1, np.float32)], axis=0)  # [6, D, F]
    w2_all = np.concatenate([np.asarray(sw2, np.float32),
                             np.asarray(rw2, np.float32)], axis=0)  # [6, F, O]
    b1_all = np.concatenate([np.asarray(sb1, np.float32),
                             np.asarray(rb1, np.float32)], axis=0)  # [6, F]
    b2_all = np.concatenate([np.asarray(sb2, np.float32),
                             np.asarray(rb2, np.float32)], axis=0)  # [6, O]

    w1_t = np.ascontiguousarray(
        w1_all.reshape(NEXP, DCH, P, F).astype(BF16))              # [6,8,128,4096]
    w2_t = np.ascontiguousarray(
        w2_all.reshape(NEXP, FCH, P, O).astype(BF16))              # [6,32,128,1024]
    b1_t = np.ascontiguousarray(
        b1_all.reshape(NEXP, FCH, P).transpose(0, 2, 1)).astype(np.float32)
    b2_t = b2_all.reshape(NEXP, 1, O).astype(BF16)
    gw_t = np.ascontiguousarray(
        np.asarray(gate_w, np.float32).reshape(DCH, P, E))
    gb_t = np.asarray(gate_b, np.float32).reshape(1, E)

    in_maps = []
    for c in range(NCORES):
        xs = xf[c * T:(c + 1) * T]                                  # [T, D]
        xt = np.ascontiguousarray(xs.T).reshape(DCH, P, T)          # [8,128,1024]
        in_maps.append({
            "x32": xt.astype(np.float32),
            "xb": xt.astype(BF16),
            "w1": w1_t, "w2": w2_t, "b1": b1_t, "b2": b2_t,
            "gw": gw_t, "gb": gb_t,
        })
    return in_maps


def kernel(**inputs) -> np.ndarray:
    nc = _get_nc()
    in_maps = _prep_inputs(**inputs)
    res = run_bass_kernel_spmd(nc, in_maps, list(range(NCORES)))
    parts = [res.results[c]["out"] for c in range(NCORES)]
    full = np.concatenate(parts, axis=0)            # [8192, 1024]
    return full.reshape(B, S, O).astype(np.float32)
